# revision 16
# baseline (speedup 1.0000x reference)
"""GQA attention kernel for 8 trn2 cores — software-pipelined v3.

Sharding: core c -> (batch c//2, head-half c%2). Each core computes a partial
out-projection for its 8 KV heads / 4 query groups on one batch; host sums the
two half partials per batch and adds bo.

Single fused pipeline paced by the Scalar-engine EXP stream (the ~273us/core
wall).  16 (group, q-tile) windows of 16 slots (one 128-row k-block each).
Per slot the PE emits: filler matmuls (K/Q/V projections for later groups,
out-projections for finished q-tiles) from a deadline-sorted queue, the PV
matmuls lagged 8 slots behind the EXP stream, and the score matmul pair for
this slot's k-block.  The pair (head 2g on PE rows 0:63 via tile_position
(0,0), head 2g+1 on rows 64:127 via (64,0)) writes one [128,1024] PSUM tile
so both matmuls are released by the same EXP and run concurrently (~109ns
each vs 216 solo).  One N=1024 EXP per k-block converts scores for both
heads into the bf16 P^T tile.  Rowsums ride the 65th column of Vones;
normalization stages the PV PSUM to SBUF (freeing the banks for the next
window), then reciprocal_approx_fast + gpsimd partition broadcast + multiply.
Biases fold into the projection PSUM->SBUF copies (tensor_scalar add with a
per-partition [128,1] bias AP), dropping the baseline's 9th contraction
chunk.  The x^T load is chunked per q-tile so the first projections start
~7us in.
"""

import numpy as np
import ml_dtypes

import concourse.bass as bass
import concourse.tile as tile
from concourse import bacc, mybir
from concourse.bass_utils import run_bass_kernel_spmd

B, S, E = 4, 2048, 1024
NH, NG, HD = 16, 8, 64
SCALE = HD ** -0.5
NCORES = 8
HG = 4                    # q-groups per core
EC = 8                    # e-chunks (contraction 1024 = 8*128)
QT = 4                    # 512-wide q tiles
KB = 16                   # 128-row k blocks
SB = 16                   # 128-row s blocks
LAG = 8                   # PV trails the score/EXP stream by LAG slots

BF = mybir.dt.bfloat16
F32 = mybir.dt.float32
ADD = mybir.AluOpType.add

_CACHE = {}
LAST_RESULT = None


def _build_program():
    from contextlib import ExitStack

    nc = bacc.Bacc("TRN2", target_bir_lowering=False, debug=False)
    x_d = nc.dram_tensor("x", [128, EC, S], BF, kind="ExternalInput").ap()
    wq_d = nc.dram_tensor("wq", [E, 512], BF, kind="ExternalInput").ap()
    wk_d = nc.dram_tensor("wk", [E, 512], BF, kind="ExternalInput").ap()
    wv_d = nc.dram_tensor("wv", [E, 512], BF, kind="ExternalInput").ap()
    wo_d = nc.dram_tensor("wo", [512, E], BF, kind="ExternalInput").ap()
    qb_d = nc.dram_tensor("qb", [128, HG], F32, kind="ExternalInput").ap()
    kb_d = nc.dram_tensor("kb", [128, HG], F32, kind="ExternalInput").ap()
    vb_d = nc.dram_tensor("vb", [128, 2, 256], F32, kind="ExternalInput").ap()
    out_d = nc.dram_tensor("out", [S, E], F32, kind="ExternalOutput").ap()

    Exp = mybir.ActivationFunctionType.Exp

    with tile.TileContext(nc) as tc, ExitStack() as ctx:
        persist = ctx.enter_context(tc.tile_pool(name="persist", bufs=1))
        ktp = ctx.enter_context(tc.tile_pool(name="ktp", bufs=2))
        ptp = ctx.enter_context(tc.tile_pool(name="ptp", bufs=2))
        small = ctx.enter_context(tc.tile_pool(name="small", bufs=1))
        otp = ctx.enter_context(tc.tile_pool(name="otp", bufs=2))
        psSC = ctx.enter_context(tc.tile_pool(name="psSC", bufs=2, space="PSUM"))
        psPV = ctx.enter_context(tc.tile_pool(name="psPV", bufs=1, space="PSUM"))
        psOP = ctx.enter_context(tc.tile_pool(name="psOP", bufs=2, space="PSUM"))

        xT = persist.tile([128, EC, S], BF, tag="xT")
        wq = persist.tile([128, EC, 512], BF, tag="wq")
        wk = persist.tile([128, EC, 512], BF, tag="wk")
        wv = persist.tile([128, EC, 512], BF, tag="wv")
        wo = persist.tile([128, 4, E], BF, tag="wo")
        qb = persist.tile([128, HG], F32, tag="qb")
        kbt = persist.tile([128, HG], F32, tag="kbt")
        vb = persist.tile([128, 2, 256], F32, tag="vb")
        Vones = persist.tile([128, KB, 8, HD + 1], BF, tag="Vones")
        aoT = persist.tile([128, 4, S], BF, tag="aoT")

        # x^T is transposed host-side; chunked load per q-tile so K(0,qt)
        # can start as soon as its chunk lands.  wk/wq first: K(0,0)/Q(0,0)
        # gate the pipeline start.
        nc.sync.dma_start(out=qb, in_=qb_d)
        nc.sync.dma_start(out=kbt, in_=kb_d)
        nc.sync.dma_start(out=vb, in_=vb_d)
        wk_r = wk_d.rearrange("(c p) n -> p c n", p=128)
        wq_r = wq_d.rearrange("(c p) n -> p c n", p=128)
        nc.sync.dma_start(out=wk[:, :, 0:128], in_=wk_r[:, :, 0:128])
        nc.sync.dma_start(out=xT[:, :, 0:512], in_=x_d[:, :, 0:512])
        nc.sync.dma_start(out=wq[:, :, 0:128], in_=wq_r[:, :, 0:128])
        nc.sync.dma_start(out=wk[:, :, 128:512], in_=wk_r[:, :, 128:512])
        nc.sync.dma_start(out=wq[:, :, 128:512], in_=wq_r[:, :, 128:512])
        nc.sync.dma_start(out=wv, in_=wv_d.rearrange("(c p) n -> p c n", p=128))
        for qt in range(1, QT):
            qs = slice(qt * 512, (qt + 1) * 512)
            nc.sync.dma_start(out=xT[:, :, qs], in_=x_d[:, :, qs])
        nc.sync.dma_start(out=wo, in_=wo_d.rearrange("(c p) n -> p c n", p=128))
        nc.vector.memset(Vones[:, :, :, HD:HD + 1], 1.0)

        KTt: dict[int, tile.Tile] = {}
        QTt: dict[int, tile.Tile] = {}

        # ---------- emission helpers ----------
        def proj_kq(which: str, g: int, qt: int):
            qs = slice(qt * 512, (qt + 1) * 512)
            if which == "k":
                if qt == 0:
                    KTt[g] = ktp.tile([128, S], BF, tag="KT", name=f"KT{g}")
                dst, w, bias = KTt[g], wk, kbt
            else:
                if qt == 0:
                    QTt[g] = ktp.tile([128, S], BF, tag="QT", name=f"QT{g}")
                dst, w, bias = QTt[g], wq, qb
            ps = psOP.tile([128, 512], F32, tag="op")
            for c in range(EC):
                nc.tensor.matmul(ps, lhsT=w[:, c, g * 128:(g + 1) * 128],
                                 rhs=xT[:, c, qs],
                                 start=(c == 0), stop=(c == EC - 1))
            nc.vector.tensor_scalar(
                out=dst[:, qs], in0=ps, scalar1=bias[:, g:g + 1], scalar2=None,
                op0=ADD)

        def proj_v(h01: int, sb: int):
            ss = slice(sb * 128, (sb + 1) * 128)
            ps = psOP.tile([128, 512], F32, tag="op")
            for c in range(EC):
                nc.tensor.matmul(ps[:, 0:256], lhsT=xT[:, c, ss],
                                 rhs=wv[:, c, h01 * 256:(h01 + 1) * 256],
                                 start=(c == 0), stop=(c == EC - 1))
            nc.vector.tensor_add(
                out=Vones[:, sb, 4 * h01:4 * h01 + 4, 0:HD],
                in0=ps[:, 0:256].rearrange("p (h d) -> p h d", h=4),
                in1=vb[:, h01, :].rearrange("p (h d) -> p h d", h=4))

        def outproj_item(qt: int, sb4: int, et: int):
            sb = qt * 4 + sb4
            ss = slice(sb * 128, (sb + 1) * 128)
            es = slice(et * 512, (et + 1) * 512)
            po = psOP.tile([128, 512], F32, tag="op")
            for c in range(4):
                nc.tensor.matmul(po, lhsT=aoT[:, c, ss], rhs=wo[:, c, es],
                                 start=(c == 0), stop=(c == 3))
            ot = otp.tile([128, 512], F32, tag="ot")
            nc.vector.tensor_copy(out=ot, in_=po)
            nc.sync.dma_start(out=out_d[ss, es], in_=ot)

        def scores_slot(g: int, qt: int, kb: int, pt):
            qs = slice(qt * 512, (qt + 1) * 512)
            ks = slice(kb * 128, (kb + 1) * 128)
            sc = psSC.tile([128, 1024], F32, tag="sc")
            nc.tensor.matmul(sc[:, 0:512], lhsT=KTt[g][0:64, ks],
                             rhs=QTt[g][0:64, qs], start=True, stop=True,
                             tile_position=(0, 0))
            nc.tensor.matmul(sc[:, 512:1024], lhsT=KTt[g][64:128, ks],
                             rhs=QTt[g][64:128, qs], start=True, stop=True,
                             tile_position=(64, 0))
            nc.scalar.activation(
                out=pt[:, kb, :, :],
                in_=sc.rearrange("p (h q) -> p h q", h=2), func=Exp)

        def pv_slot(g: int, kb: int, pt, pva, pvb):
            nc.tensor.matmul(pva[0:HD + 1, :], lhsT=Vones[:, kb, 2 * g, :],
                             rhs=pt[:, kb, 0, :],
                             start=(kb == 0), stop=(kb == KB - 1))
            nc.tensor.matmul(pvb[0:HD + 1, :],
                             lhsT=Vones[:, kb, 2 * g + 1, :],
                             rhs=pt[:, kb, 1, :],
                             start=(kb == 0), stop=(kb == KB - 1))

        def norm(g: int, qt: int, pva, pvb):
            qs = slice(qt * 512, (qt + 1) * 512)
            # stage PSUM->SBUF first so the PV banks free quickly
            stA = small.tile([HD + 1, 512], F32, tag="stA")
            nc.vector.tensor_copy(out=stA, in_=pva[0:HD + 1, :])
            stB = small.tile([HD + 1, 512], F32, tag="stB")
            nc.vector.tensor_copy(out=stB, in_=pvb[0:HD + 1, :])
            rs0 = small.tile([1, 2, 512], F32, tag="rs0")
            nc.vector.tensor_copy(out=rs0[0:1, 0, :], in_=stA[HD:HD + 1, :])
            nc.vector.tensor_copy(out=rs0[0:1, 1, :], in_=stB[HD:HD + 1, :])
            rsi = small.tile([1, 2, 512], F32, tag="rsi")
            nc.vector.reciprocal_approx_fast(out=rsi, in_=rs0)
            repA = small.tile([64, 512], F32, tag="repA")
            nc.gpsimd.partition_broadcast(out_ap=repA, in_ap=rsi[0:1, 0, :])
            repB = small.tile([64, 512], F32, tag="repB")
            nc.gpsimd.partition_broadcast(out_ap=repB, in_ap=rsi[0:1, 1, :])
            nc.vector.tensor_mul(out=aoT[0:64, g, qs], in0=stA[0:HD, :],
                                 in1=repA)
            nc.vector.tensor_mul(out=aoT[64:128, g, qs], in0=stB[0:HD, :],
                                 in1=repB)

        # ---------- filler queue (deadline-sorted) ----------
        # deadline = (window, slot) BEFORE which the item must be emitted.
        C_KQ, C_V, C_OP = 2.0, 1.2, 0.9
        items = []
        for qt in range(1, 4):
            items.append(((0, 4 * qt - 1), C_KQ,
                          lambda qt=qt: proj_kq("k", 0, qt)))
        for sb in range(SB):
            dl = (0, LAG + sb) if sb < 8 else (1, sb - 8)
            items.append((dl, C_V, lambda sb=sb: proj_v(0, sb)))
        for qt in range(1, 4):
            items.append(((qt, 0), C_KQ, lambda qt=qt: proj_kq("q", 0, qt)))
        for g in range(1, 4):
            for qt in range(4):
                items.append(((4 * g, 0), C_KQ,
                              lambda g=g, qt=qt: proj_kq("k", g, qt)))
            items.append(((4 * g, 0), C_KQ,
                          lambda g=g: proj_kq("q", g, 0)))
            for qt in range(1, 4):
                items.append(((4 * g + qt, 0), C_KQ,
                              lambda g=g, qt=qt: proj_kq("q", g, qt)))
        for sb in range(SB):
            dl = (8, LAG + sb) if sb < 8 else (9, sb - 8)
            items.append((dl, C_V, lambda sb=sb: proj_v(1, sb)))
        items.sort(key=lambda it: it[0])
        queue = list(items)

        def drain(w, j, opt_budget, spent):
            while queue:
                dl, cost, fn = queue[0]
                forced = dl <= (w, j)
                if not forced and spent[0] + cost > opt_budget:
                    break
                queue.pop(0)
                fn()
                spent[0] += cost

        # ---------- startup ----------
        # dummy matmuls warm the PE HAM clock gate (1.2 -> 2.4 GHz) while
        # the first DMAs stream in; results are never read.
        warm = psSC.tile([128, 1024], F32, tag="sc", name="warm")
        for i in range(40):
            nc.tensor.matmul(warm[:, 0:512], lhsT=Vones[:, 0, :, :].rearrange(
                "p h d -> p (h d)")[:, 0:128],
                rhs=Vones[:, 1, :, :].rearrange("p h d -> p (h d)")[:, 0:512],
                start=True, stop=True)
        proj_kq("k", 0, 0)
        proj_kq("q", 0, 0)

        # ---------- pipelined windows ----------
        # window w (= t): slots 0..15; slot j emits:
        #   fillers / outproj; PV(t-1, kb=j+8) for j<8 else PV(t, kb=j-8);
        #   scores+EXP for (t, kb=j)
        prev = None   # (g, qt, pt) of previous window
        pvcur = None  # (g, qt, pva, pvb) accumulation in flight
        # out-proj for qt becomes legal after norm(3, qt) at window 13+qt
        # slot 7; gate each item on (13+qt, 8).
        op_queue = [((13 + qt, 8), (qt, sb4, et))
                    for qt in range(3)
                    for et in range(2) for sb4 in range(4)]
        for w in range(16):
            g, qt = w // 4, w % 4
            pt = ptp.tile([128, KB, 2, 512], BF, tag="pt")
            fixed = 3.6 if prev is None else 10.6
            opt_budget = max(0.0, 16.6 - fixed)
            spent = [0.0]
            for j in range(16):
                slot_allow = opt_budget * (j + 1) / 16
                drain(w, j, slot_allow, spent)
                while (op_queue and op_queue[0][0] <= (w, j)
                       and spent[0] + C_OP <= slot_allow + 2.5):
                    _, o = op_queue.pop(0)
                    outproj_item(*o)
                    spent[0] += C_OP
                if j < 8:
                    if prev is not None:
                        pv_slot(prev[0], j + 8, prev[2], pvcur[2], pvcur[3])
                        if j == 7:
                            norm(prev[0], prev[1], pvcur[2], pvcur[3])
                else:
                    if j == 8:
                        pva = psPV.tile([128, 512], F32, tag="pva")
                        pvb = psPV.tile([128, 512], F32, tag="pvb")
                        pvcur = (g, qt, pva, pvb)
                    pv_slot(g, j - 8, pt, pvcur[2], pvcur[3])
                if j % 2 == 0:
                    scores_slot(g, qt, j, pt)
                    scores_slot(g, qt, j + 1, pt)
            prev = (g, qt, pt)

        # ---------- tail ----------
        for kb in range(8, 16):
            pv_slot(prev[0], kb, prev[2], pvcur[2], pvcur[3])
        norm(prev[0], prev[1], pvcur[2], pvcur[3])
        assert not queue
        for _, o in op_queue:
            outproj_item(*o)
        for et in range(2):
            for sb4 in range(4):
                outproj_item(3, sb4, et)

    nc.compile()
    return nc


def _prep_shards(x, Wq, bq, Wk, bk, Wv, bv, Wo):
    """Host-side shard prep. Returns per-core input maps (bf16)."""
    bf16 = ml_dtypes.bfloat16
    # host-side transpose into the device layout [128, EC, S]
    xs = [np.ascontiguousarray(
        x[b].T.reshape(EC, 128, S).transpose(1, 0, 2)).astype(bf16)
        for b in range(B)]
    halves = []
    for half in range(2):
        # Wq: scale folded in, columns duplicated per group
        wq_cols = (Wq[:, half * 256:(half + 1) * 256] * SCALE).reshape(E, HG, HD)
        wq_f = np.concatenate([wq_cols, wq_cols], axis=2).reshape(E, 512)
        bq_h = (bq[half * 256:(half + 1) * 256] * SCALE).reshape(HG, HD)
        qb_f = np.concatenate([bq_h, bq_h], axis=1).T.copy()  # [128, HG]

        wk_f = Wk[:, half * 512:(half + 1) * 512]
        kb_f = bk[half * 512:(half + 1) * 512].reshape(HG, 128).T.copy()

        wv_f = Wv[:, half * 512:(half + 1) * 512]
        vb_f = np.broadcast_to(
            bv[half * 512:(half + 1) * 512].reshape(1, 2, 256),
            (128, 2, 256)).copy()

        wo_f = Wo[half * 512:(half + 1) * 512, :]
        halves.append({
            "wq": np.ascontiguousarray(wq_f).astype(bf16),
            "wk": np.ascontiguousarray(wk_f).astype(bf16),
            "wv": np.ascontiguousarray(wv_f).astype(bf16),
            "wo": np.ascontiguousarray(wo_f).astype(bf16),
            "qb": qb_f.astype(np.float32),
            "kb": kb_f.astype(np.float32),
            "vb": vb_f.astype(np.float32),
        })
    in_maps = []
    for c in range(NCORES):
        m = {"x": xs[c // 2]}
        m.update(halves[c % 2])
        in_maps.append(m)
    return in_maps


def kernel(x, Wq, bq, Wk, bk, Wv, bv, Wo, bo):
    global LAST_RESULT
    x, Wq, bq, Wk, bk, Wv, bv, Wo, bo = [
        np.asarray(a, dtype=np.float32)
        for a in (x, Wq, bq, Wk, bk, Wv, bv, Wo, bo)]
    if "nc" not in _CACHE:
        _CACHE["nc"] = _build_program()
    nc = _CACHE["nc"]
    in_maps = _prep_shards(x, Wq, bq, Wk, bk, Wv, bv, Wo)
    res = run_bass_kernel_spmd(nc, in_maps, core_ids=list(range(NCORES)))
    LAST_RESULT = res
    out = np.empty((B, S, E), np.float32)
    for b in range(B):
        out[b] = res.results[2 * b]["out"] + res.results[2 * b + 1]["out"]
    out += bo.astype(np.float32)
    return out


# revision 17
# speedup vs baseline: 1.1675x; 1.1675x over previous
"""GQA attention kernel for 8 trn2 cores — software-pipelined v3.

Sharding: core c -> (batch c//2, head-half c%2). Each core computes a partial
out-projection for its 8 KV heads / 4 query groups on one batch; host sums the
two half partials per batch and adds bo.

Single fused pipeline paced by the Scalar-engine EXP stream (the ~273us/core
wall).  16 (group, q-tile) windows of 16 slots (one 128-row k-block each).
Per slot the PE emits: filler matmuls (K/Q/V projections for later groups,
out-projections for finished q-tiles) from a deadline-sorted queue, the PV
matmuls lagged 8 slots behind the EXP stream, and the score matmul pair for
this slot's k-block.  The pair (head 2g on PE rows 0:63 via tile_position
(0,0), head 2g+1 on rows 64:127 via (64,0)) writes one [128,1024] PSUM tile
so both matmuls are released by the same EXP and run concurrently (~109ns
each vs 216 solo).  One N=1024 EXP per k-block converts scores for both
heads into the bf16 P^T tile.  Rowsums ride the 65th column of Vones;
normalization stages the PV PSUM to SBUF (freeing the banks for the next
window), then reciprocal_approx_fast + gpsimd partition broadcast + multiply.
Biases fold into the projection PSUM->SBUF copies (tensor_scalar add with a
per-partition [128,1] bias AP), dropping the baseline's 9th contraction
chunk.  The x^T load is chunked per q-tile so the first projections start
~7us in.
"""

import numpy as np
import ml_dtypes

import concourse.bass as bass
import concourse.tile as tile
from concourse import bacc, mybir
from concourse.bass_utils import run_bass_kernel_spmd

B, S, E = 4, 2048, 1024
NH, NG, HD = 16, 8, 64
SCALE = HD ** -0.5
NCORES = 8
HG = 4                    # q-groups per core
EC = 8                    # e-chunks (contraction 1024 = 8*128)
QT = 4                    # 512-wide q tiles
KB = 16                   # 128-row k blocks
SB = 16                   # 128-row s blocks
LAG = 8                   # PV trails the score/EXP stream by LAG slots

BF = mybir.dt.bfloat16
F32 = mybir.dt.float32
ADD = mybir.AluOpType.add

_CACHE = {}
LAST_RESULT = None


def _build_program():
    from contextlib import ExitStack

    nc = bacc.Bacc("TRN2", target_bir_lowering=False, debug=False)
    x_d = nc.dram_tensor("x", [128, EC, S], BF, kind="ExternalInput").ap()
    wq_d = nc.dram_tensor("wq", [E, 512], BF, kind="ExternalInput").ap()
    wk_d = nc.dram_tensor("wk", [E, 512], BF, kind="ExternalInput").ap()
    wv_d = nc.dram_tensor("wv", [E, 512], BF, kind="ExternalInput").ap()
    wo_d = nc.dram_tensor("wo", [512, E], BF, kind="ExternalInput").ap()
    qb_d = nc.dram_tensor("qb", [128, HG], F32, kind="ExternalInput").ap()
    kb_d = nc.dram_tensor("kb", [128, HG], F32, kind="ExternalInput").ap()
    vb_d = nc.dram_tensor("vb", [128, 2, 256], F32, kind="ExternalInput").ap()
    out_d = nc.dram_tensor("out", [S, E], F32, kind="ExternalOutput").ap()

    Exp = mybir.ActivationFunctionType.Exp

    with tile.TileContext(nc) as tc, ExitStack() as ctx:
        persist = ctx.enter_context(tc.tile_pool(name="persist", bufs=1))
        ktp = ctx.enter_context(tc.tile_pool(name="ktp", bufs=2))
        ptp = ctx.enter_context(tc.tile_pool(name="ptp", bufs=2))
        small = ctx.enter_context(tc.tile_pool(name="small", bufs=1))
        otp = ctx.enter_context(tc.tile_pool(name="otp", bufs=2))
        psSC = ctx.enter_context(tc.tile_pool(name="psSC", bufs=2, space="PSUM"))
        psPV = ctx.enter_context(tc.tile_pool(name="psPV", bufs=1, space="PSUM"))
        psOP = ctx.enter_context(tc.tile_pool(name="psOP", bufs=2, space="PSUM"))

        xT = persist.tile([128, EC, S], BF, tag="xT")
        wq = persist.tile([128, EC, 512], BF, tag="wq")
        wk = persist.tile([128, EC, 512], BF, tag="wk")
        wv = persist.tile([128, EC, 512], BF, tag="wv")
        wo = persist.tile([128, 4, E], BF, tag="wo")
        qb = persist.tile([128, HG], F32, tag="qb")
        kbt = persist.tile([128, HG], F32, tag="kbt")
        vb = persist.tile([128, 2, 256], F32, tag="vb")
        Vones = persist.tile([128, KB, 8, HD + 1], BF, tag="Vones")
        aoT = persist.tile([128, 4, S], BF, tag="aoT")

        # x^T is transposed host-side; chunked load per q-tile so K(0,qt)
        # can start as soon as its chunk lands.  wk/wq first: K(0,0)/Q(0,0)
        # gate the pipeline start.
        nc.sync.dma_start(out=qb, in_=qb_d)
        nc.sync.dma_start(out=kbt, in_=kb_d)
        nc.sync.dma_start(out=vb, in_=vb_d)
        wk_r = wk_d.rearrange("(c p) n -> p c n", p=128)
        wq_r = wq_d.rearrange("(c p) n -> p c n", p=128)
        nc.sync.dma_start(out=wk[:, :, 0:128], in_=wk_r[:, :, 0:128])
        nc.sync.dma_start(out=xT[:, :, 0:512], in_=x_d[:, :, 0:512])
        nc.sync.dma_start(out=wq[:, :, 0:128], in_=wq_r[:, :, 0:128])
        nc.sync.dma_start(out=wk[:, :, 128:512], in_=wk_r[:, :, 128:512])
        nc.sync.dma_start(out=wq[:, :, 128:512], in_=wq_r[:, :, 128:512])
        nc.sync.dma_start(out=wv, in_=wv_d.rearrange("(c p) n -> p c n", p=128))
        for qt in range(1, QT):
            qs = slice(qt * 512, (qt + 1) * 512)
            nc.sync.dma_start(out=xT[:, :, qs], in_=x_d[:, :, qs])
        nc.sync.dma_start(out=wo, in_=wo_d.rearrange("(c p) n -> p c n", p=128))
        nc.vector.memset(Vones[:, :, :, HD:HD + 1], 1.0)

        KTt: dict[int, tile.Tile] = {}
        QTt: dict[int, tile.Tile] = {}

        # ---------- emission helpers ----------
        def proj_kq(which: str, g: int, qt: int):
            qs = slice(qt * 512, (qt + 1) * 512)
            if which == "k":
                if qt == 0:
                    KTt[g] = ktp.tile([128, S], BF, tag="KT", name=f"KT{g}")
                dst, w, bias = KTt[g], wk, kbt
            else:
                if qt == 0:
                    QTt[g] = ktp.tile([128, S], BF, tag="QT", name=f"QT{g}")
                dst, w, bias = QTt[g], wq, qb
            ps = psOP.tile([128, 512], F32, tag="op")
            for c in range(EC):
                nc.tensor.matmul(ps, lhsT=w[:, c, g * 128:(g + 1) * 128],
                                 rhs=xT[:, c, qs],
                                 start=(c == 0), stop=(c == EC - 1))
            nc.vector.tensor_scalar(
                out=dst[:, qs], in0=ps, scalar1=bias[:, g:g + 1], scalar2=None,
                op0=ADD)

        def proj_v(h01: int, sb: int):
            ss = slice(sb * 128, (sb + 1) * 128)
            ps = psOP.tile([128, 512], F32, tag="op")
            for c in range(EC):
                nc.tensor.matmul(ps[:, 0:256], lhsT=xT[:, c, ss],
                                 rhs=wv[:, c, h01 * 256:(h01 + 1) * 256],
                                 start=(c == 0), stop=(c == EC - 1))
            nc.vector.tensor_add(
                out=Vones[:, sb, 4 * h01:4 * h01 + 4, 0:HD],
                in0=ps[:, 0:256].rearrange("p (h d) -> p h d", h=4),
                in1=vb[:, h01, :].rearrange("p (h d) -> p h d", h=4))

        def outproj_item(qt: int, sb4: int, et: int):
            sb = qt * 4 + sb4
            ss = slice(sb * 128, (sb + 1) * 128)
            es = slice(et * 512, (et + 1) * 512)
            po = psOP.tile([128, 512], F32, tag="op")
            for c in range(4):
                nc.tensor.matmul(po, lhsT=aoT[:, c, ss], rhs=wo[:, c, es],
                                 start=(c == 0), stop=(c == 3))
            ot = otp.tile([128, 512], F32, tag="ot")
            nc.vector.tensor_copy(out=ot, in_=po)
            nc.sync.dma_start(out=out_d[ss, es], in_=ot)

        def scores_slot(g: int, qt: int, kb: int, pt):
            qs = slice(qt * 512, (qt + 1) * 512)
            ks = slice(kb * 128, (kb + 1) * 128)
            sc = psSC.tile([128, 1024], F32, tag="sc")
            nc.tensor.matmul(sc[:, 0:512], lhsT=KTt[g][0:64, ks],
                             rhs=QTt[g][0:64, qs], start=True, stop=True,
                             tile_position=(0, 0))
            nc.tensor.matmul(sc[:, 512:1024], lhsT=KTt[g][64:128, ks],
                             rhs=QTt[g][64:128, qs], start=True, stop=True,
                             tile_position=(64, 0))
            nc.scalar.activation(
                out=pt[:, kb, :, :],
                in_=sc.rearrange("p (h q) -> p h q", h=2), func=Exp)

        def pv_slot(g: int, kb: int, pt, pva, pvb):
            nc.tensor.matmul(pva[0:HD + 1, :], lhsT=Vones[:, kb, 2 * g, :],
                             rhs=pt[:, kb, 0, :],
                             start=(kb == 0), stop=(kb == KB - 1))
            nc.tensor.matmul(pvb[0:HD + 1, :],
                             lhsT=Vones[:, kb, 2 * g + 1, :],
                             rhs=pt[:, kb, 1, :],
                             start=(kb == 0), stop=(kb == KB - 1))

        def norm(g: int, qt: int, pva, pvb):
            qs = slice(qt * 512, (qt + 1) * 512)
            # stage PSUM->SBUF first so the PV banks free quickly
            stA = small.tile([HD + 1, 512], F32, tag="stA")
            nc.vector.tensor_copy(out=stA, in_=pva[0:HD + 1, :])
            stB = small.tile([HD + 1, 512], F32, tag="stB")
            nc.vector.tensor_copy(out=stB, in_=pvb[0:HD + 1, :])
            rs0 = small.tile([1, 2, 512], F32, tag="rs0")
            nc.vector.tensor_copy(out=rs0[0:1, 0, :], in_=stA[HD:HD + 1, :])
            nc.vector.tensor_copy(out=rs0[0:1, 1, :], in_=stB[HD:HD + 1, :])
            rsi = small.tile([1, 2, 512], F32, tag="rsi")
            nc.vector.reciprocal_approx_fast(out=rsi, in_=rs0)
            repA = small.tile([64, 512], F32, tag="repA")
            nc.gpsimd.partition_broadcast(out_ap=repA, in_ap=rsi[0:1, 0, :])
            repB = small.tile([64, 512], F32, tag="repB")
            nc.gpsimd.partition_broadcast(out_ap=repB, in_ap=rsi[0:1, 1, :])
            nc.vector.tensor_mul(out=aoT[0:64, g, qs], in0=stA[0:HD, :],
                                 in1=repA)
            nc.vector.tensor_mul(out=aoT[64:128, g, qs], in0=stB[0:HD, :],
                                 in1=repB)

        # ---------- filler queue (deadline-sorted) ----------
        # deadline = (window, slot) BEFORE which the item must be emitted.
        C_KQ, C_V, C_OP = 2.0, 1.2, 0.9
        items = []
        for qt in range(1, 4):
            items.append(((0, 4 * qt - 1), C_KQ,
                          lambda qt=qt: proj_kq("k", 0, qt)))
        for sb in range(SB):
            dl = (0, LAG + sb) if sb < 8 else (1, sb - 8)
            items.append((dl, C_V, lambda sb=sb: proj_v(0, sb)))
        for qt in range(1, 4):
            items.append(((qt, 0), C_KQ, lambda qt=qt: proj_kq("q", 0, qt)))
        for g in range(1, 4):
            for qt in range(4):
                items.append(((4 * g, 0), C_KQ,
                              lambda g=g, qt=qt: proj_kq("k", g, qt)))
            items.append(((4 * g, 0), C_KQ,
                          lambda g=g: proj_kq("q", g, 0)))
            for qt in range(1, 4):
                items.append(((4 * g + qt, 0), C_KQ,
                              lambda g=g, qt=qt: proj_kq("q", g, qt)))
        for sb in range(SB):
            dl = (8, LAG + sb) if sb < 8 else (9, sb - 8)
            items.append((dl, C_V, lambda sb=sb: proj_v(1, sb)))
        items.sort(key=lambda it: it[0])
        queue = list(items)

        def drain(w, j, opt_budget, spent):
            while queue:
                dl, cost, fn = queue[0]
                forced = dl <= (w, j)
                if not forced and spent[0] + cost > opt_budget:
                    break
                queue.pop(0)
                fn()
                spent[0] += cost

        # ---------- startup ----------
        proj_kq("k", 0, 0)
        proj_kq("q", 0, 0)

        # ---------- pipelined windows ----------
        # window w (= t): slots 0..15; slot j emits:
        #   fillers / outproj; PV(t-1, kb=j+8) for j<8 else PV(t, kb=j-8);
        #   scores+EXP for (t, kb=j)
        prev = None   # (g, qt, pt) of previous window
        pvcur = None  # (g, qt, pva, pvb) accumulation in flight
        # out-proj for qt becomes legal after norm(3, qt) at window 13+qt
        # slot 7; gate each item on (13+qt, 8).
        op_queue = [((13 + qt, 8), (qt, sb4, et))
                    for qt in range(3)
                    for et in range(2) for sb4 in range(4)]
        for w in range(16):
            g, qt = w // 4, w % 4
            pt = ptp.tile([128, KB, 2, 512], BF, tag="pt")
            fixed = 3.6 if prev is None else 10.6
            opt_budget = max(0.0, 16.6 - fixed)
            spent = [0.0]
            for j in range(16):
                slot_allow = opt_budget * (j + 1) / 16
                drain(w, j, slot_allow, spent)
                while (op_queue and op_queue[0][0] <= (w, j)
                       and spent[0] + C_OP <= slot_allow + 2.5):
                    _, o = op_queue.pop(0)
                    outproj_item(*o)
                    spent[0] += C_OP
                if j < 8:
                    if prev is not None:
                        pv_slot(prev[0], j + 8, prev[2], pvcur[2], pvcur[3])
                        if j == 7:
                            norm(prev[0], prev[1], pvcur[2], pvcur[3])
                else:
                    if j == 8:
                        pva = psPV.tile([128, 512], F32, tag="pva")
                        pvb = psPV.tile([128, 512], F32, tag="pvb")
                        pvcur = (g, qt, pva, pvb)
                    pv_slot(g, j - 8, pt, pvcur[2], pvcur[3])
                if j % 2 == 0:
                    scores_slot(g, qt, j, pt)
                    scores_slot(g, qt, j + 1, pt)
            prev = (g, qt, pt)

        # ---------- tail ----------
        for kb in range(8, 16):
            pv_slot(prev[0], kb, prev[2], pvcur[2], pvcur[3])
        norm(prev[0], prev[1], pvcur[2], pvcur[3])
        assert not queue
        for _, o in op_queue:
            outproj_item(*o)
        for et in range(2):
            for sb4 in range(4):
                outproj_item(3, sb4, et)

    nc.compile()
    return nc


def _prep_shards(x, Wq, bq, Wk, bk, Wv, bv, Wo):
    """Host-side shard prep. Returns per-core input maps (bf16)."""
    bf16 = ml_dtypes.bfloat16
    # host-side transpose into the device layout [128, EC, S]
    xs = [np.ascontiguousarray(
        x[b].T.reshape(EC, 128, S).transpose(1, 0, 2)).astype(bf16)
        for b in range(B)]
    halves = []
    for half in range(2):
        # Wq: scale folded in, columns duplicated per group
        wq_cols = (Wq[:, half * 256:(half + 1) * 256] * SCALE).reshape(E, HG, HD)
        wq_f = np.concatenate([wq_cols, wq_cols], axis=2).reshape(E, 512)
        bq_h = (bq[half * 256:(half + 1) * 256] * SCALE).reshape(HG, HD)
        qb_f = np.concatenate([bq_h, bq_h], axis=1).T.copy()  # [128, HG]

        wk_f = Wk[:, half * 512:(half + 1) * 512]
        kb_f = bk[half * 512:(half + 1) * 512].reshape(HG, 128).T.copy()

        wv_f = Wv[:, half * 512:(half + 1) * 512]
        vb_f = np.broadcast_to(
            bv[half * 512:(half + 1) * 512].reshape(1, 2, 256),
            (128, 2, 256)).copy()

        wo_f = Wo[half * 512:(half + 1) * 512, :]
        halves.append({
            "wq": np.ascontiguousarray(wq_f).astype(bf16),
            "wk": np.ascontiguousarray(wk_f).astype(bf16),
            "wv": np.ascontiguousarray(wv_f).astype(bf16),
            "wo": np.ascontiguousarray(wo_f).astype(bf16),
            "qb": qb_f.astype(np.float32),
            "kb": kb_f.astype(np.float32),
            "vb": vb_f.astype(np.float32),
        })
    in_maps = []
    for c in range(NCORES):
        m = {"x": xs[c // 2]}
        m.update(halves[c % 2])
        in_maps.append(m)
    return in_maps


def kernel(x, Wq, bq, Wk, bk, Wv, bv, Wo, bo):
    global LAST_RESULT
    x, Wq, bq, Wk, bk, Wv, bv, Wo, bo = [
        np.asarray(a, dtype=np.float32)
        for a in (x, Wq, bq, Wk, bk, Wv, bv, Wo, bo)]
    if "nc" not in _CACHE:
        _CACHE["nc"] = _build_program()
    nc = _CACHE["nc"]
    in_maps = _prep_shards(x, Wq, bq, Wk, bk, Wv, bv, Wo)
    res = run_bass_kernel_spmd(nc, in_maps, core_ids=list(range(NCORES)))
    LAST_RESULT = res
    out = np.empty((B, S, E), np.float32)
    for b in range(B):
        out[b] = res.results[2 * b]["out"] + res.results[2 * b + 1]["out"]
    out += bo.astype(np.float32)
    return out


# revision 19
# speedup vs baseline: 1.1993x; 1.0273x over previous
"""GQA attention kernel for 8 trn2 cores — software-pipelined v3.

Sharding: core c -> (batch c//2, head-half c%2). Each core computes a partial
out-projection for its 8 KV heads / 4 query groups on one batch; host sums the
two half partials per batch and adds bo.

Single fused pipeline paced by the Scalar-engine EXP stream (the ~273us/core
wall).  16 (group, q-tile) windows of 16 slots (one 128-row k-block each).
Per slot the PE emits: filler matmuls (K/Q/V projections for later groups,
out-projections for finished q-tiles) from a deadline-sorted queue, the PV
matmuls lagged 8 slots behind the EXP stream, and the score matmul pair for
this slot's k-block.  The pair (head 2g on PE rows 0:63 via tile_position
(0,0), head 2g+1 on rows 64:127 via (64,0)) writes one [128,1024] PSUM tile
so both matmuls are released by the same EXP and run concurrently (~109ns
each vs 216 solo).  One N=1024 EXP per k-block converts scores for both
heads into the bf16 P^T tile.  Rowsums ride the 65th column of Vones;
normalization stages the PV PSUM to SBUF (freeing the banks for the next
window), then reciprocal_approx_fast + gpsimd partition broadcast + multiply.
Biases fold into the projection PSUM->SBUF copies (tensor_scalar add with a
per-partition [128,1] bias AP), dropping the baseline's 9th contraction
chunk.  The x^T load is chunked per q-tile so the first projections start
~7us in.
"""

import numpy as np
import ml_dtypes

import concourse.bass as bass
import concourse.tile as tile
from concourse import bacc, mybir
from concourse.bass_utils import run_bass_kernel_spmd

B, S, E = 4, 2048, 1024
NH, NG, HD = 16, 8, 64
SCALE = HD ** -0.5
NCORES = 8
HG = 4                    # q-groups per core
EC = 8                    # e-chunks (contraction 1024 = 8*128)
QT = 4                    # 512-wide q tiles
KB = 16                   # 128-row k blocks
SB = 16                   # 128-row s blocks
LAG = 8                   # PV trails the score/EXP stream by LAG slots

BF = mybir.dt.bfloat16
F32 = mybir.dt.float32
ADD = mybir.AluOpType.add

_CACHE = {}
LAST_RESULT = None


def _build_program():
    from contextlib import ExitStack

    nc = bacc.Bacc("TRN2", target_bir_lowering=False, debug=False)
    x_d = nc.dram_tensor("x", [128, EC, S], BF, kind="ExternalInput").ap()
    wq_d = nc.dram_tensor("wq", [E, 512], BF, kind="ExternalInput").ap()
    wk_d = nc.dram_tensor("wk", [E, 512], BF, kind="ExternalInput").ap()
    wv_d = nc.dram_tensor("wv", [E, 512], BF, kind="ExternalInput").ap()
    wo_d = nc.dram_tensor("wo", [512, E], BF, kind="ExternalInput").ap()
    qb_d = nc.dram_tensor("qb", [128, HG], F32, kind="ExternalInput").ap()
    kb_d = nc.dram_tensor("kb", [128, HG], F32, kind="ExternalInput").ap()
    vb_d = nc.dram_tensor("vb", [128, 2, 256], F32, kind="ExternalInput").ap()
    out_d = nc.dram_tensor("out", [S, E], F32, kind="ExternalOutput").ap()

    Exp = mybir.ActivationFunctionType.Exp

    with tile.TileContext(nc) as tc, ExitStack() as ctx:
        persist = ctx.enter_context(tc.tile_pool(name="persist", bufs=1))
        ktp = ctx.enter_context(tc.tile_pool(name="ktp", bufs=2))
        ptp = ctx.enter_context(tc.tile_pool(name="ptp", bufs=2))
        small = ctx.enter_context(tc.tile_pool(name="small", bufs=1))
        otp = ctx.enter_context(tc.tile_pool(name="otp", bufs=2))
        psSC = ctx.enter_context(tc.tile_pool(name="psSC", bufs=2, space="PSUM"))
        psPV = ctx.enter_context(tc.tile_pool(name="psPV", bufs=1, space="PSUM"))
        psOP = ctx.enter_context(tc.tile_pool(name="psOP", bufs=2, space="PSUM"))

        xT = persist.tile([128, EC, S], BF, tag="xT")
        wq = persist.tile([128, EC, 512], BF, tag="wq")
        wk = persist.tile([128, EC, 512], BF, tag="wk")
        wv = persist.tile([128, EC, 512], BF, tag="wv")
        wo = persist.tile([128, 4, E], BF, tag="wo")
        qb = persist.tile([128, HG], F32, tag="qb")
        kbt = persist.tile([128, HG], F32, tag="kbt")
        vb = persist.tile([128, 2, 256], F32, tag="vb")
        Vones = persist.tile([128, KB, 8, HD + 1], BF, tag="Vones")
        aoT = persist.tile([128, 4, S], BF, tag="aoT")

        # x^T is transposed host-side; chunked load per q-tile so K(0,qt)
        # can start as soon as its chunk lands.  wk/wq first: K(0,0)/Q(0,0)
        # gate the pipeline start.
        nc.sync.dma_start(out=qb, in_=qb_d)
        nc.sync.dma_start(out=kbt, in_=kb_d)
        nc.sync.dma_start(out=vb, in_=vb_d)
        wk_r = wk_d.rearrange("(c p) n -> p c n", p=128)
        wq_r = wq_d.rearrange("(c p) n -> p c n", p=128)
        nc.sync.dma_start(out=wk[:, :, 0:128], in_=wk_r[:, :, 0:128])
        nc.sync.dma_start(out=xT[:, :, 0:512], in_=x_d[:, :, 0:512])
        nc.sync.dma_start(out=wq[:, :, 0:128], in_=wq_r[:, :, 0:128])
        nc.sync.dma_start(out=wk[:, :, 128:512], in_=wk_r[:, :, 128:512])
        nc.sync.dma_start(out=wq[:, :, 128:512], in_=wq_r[:, :, 128:512])
        nc.sync.dma_start(out=wv, in_=wv_d.rearrange("(c p) n -> p c n", p=128))
        for qt in range(1, QT):
            qs = slice(qt * 512, (qt + 1) * 512)
            nc.sync.dma_start(out=xT[:, :, qs], in_=x_d[:, :, qs])
        nc.sync.dma_start(out=wo, in_=wo_d.rearrange("(c p) n -> p c n", p=128))
        nc.vector.memset(Vones[:, :, :, HD:HD + 1], 1.0)

        KTt: dict[int, tile.Tile] = {}
        QTt: dict[int, tile.Tile] = {}

        # ---------- emission helpers ----------
        def proj_kq(which: str, g: int, qt: int):
            qs = slice(qt * 512, (qt + 1) * 512)
            if which == "k":
                if qt == 0:
                    KTt[g] = ktp.tile([128, S], BF, tag="KT", name=f"KT{g}")
                dst, w, bias = KTt[g], wk, kbt
            else:
                if qt == 0:
                    QTt[g] = ktp.tile([128, S], BF, tag="QT", name=f"QT{g}")
                dst, w, bias = QTt[g], wq, qb
            ps = psOP.tile([128, 512], F32, tag="op")
            for c in range(EC):
                nc.tensor.matmul(ps, lhsT=w[:, c, g * 128:(g + 1) * 128],
                                 rhs=xT[:, c, qs],
                                 start=(c == 0), stop=(c == EC - 1))
            nc.vector.tensor_scalar(
                out=dst[:, qs], in0=ps, scalar1=bias[:, g:g + 1], scalar2=None,
                op0=ADD)

        def proj_v(h01: int, sb: int):
            ss = slice(sb * 128, (sb + 1) * 128)
            ps = psOP.tile([128, 512], F32, tag="op")
            for c in range(EC):
                nc.tensor.matmul(ps[:, 0:256], lhsT=xT[:, c, ss],
                                 rhs=wv[:, c, h01 * 256:(h01 + 1) * 256],
                                 start=(c == 0), stop=(c == EC - 1))
            nc.vector.tensor_add(
                out=Vones[:, sb, 4 * h01:4 * h01 + 4, 0:HD],
                in0=ps[:, 0:256].rearrange("p (h d) -> p h d", h=4),
                in1=vb[:, h01, :].rearrange("p (h d) -> p h d", h=4))

        def outproj_item(qt: int, sb4: int, et: int):
            sb = qt * 4 + sb4
            ss = slice(sb * 128, (sb + 1) * 128)
            es = slice(et * 512, (et + 1) * 512)
            po = psOP.tile([128, 512], F32, tag="op")
            for c in range(4):
                nc.tensor.matmul(po, lhsT=aoT[:, c, ss], rhs=wo[:, c, es],
                                 start=(c == 0), stop=(c == 3))
            ot = otp.tile([128, 512], F32, tag="ot")
            nc.vector.tensor_copy(out=ot, in_=po)
            nc.sync.dma_start(out=out_d[ss, es], in_=ot)

        def scores_slot(g: int, qt: int, kb: int, pt):
            qs = slice(qt * 512, (qt + 1) * 512)
            ks = slice(kb * 128, (kb + 1) * 128)
            sc = psSC.tile([128, 1024], F32, tag="sc")
            nc.tensor.matmul(sc[:, 0:512], lhsT=KTt[g][0:64, ks],
                             rhs=QTt[g][0:64, qs], start=True, stop=True,
                             tile_position=(0, 0))
            nc.tensor.matmul(sc[:, 512:1024], lhsT=KTt[g][64:128, ks],
                             rhs=QTt[g][64:128, qs], start=True, stop=True,
                             tile_position=(64, 0))
            nc.scalar.activation(
                out=pt[:, kb, :, :],
                in_=sc.rearrange("p (h q) -> p h q", h=2), func=Exp)

        def pv_slot(g: int, kb: int, pt, pva, pvb):
            nc.tensor.matmul(pva[0:HD + 1, :], lhsT=Vones[:, kb, 2 * g, :],
                             rhs=pt[:, kb, 0, :],
                             start=(kb == 0), stop=(kb == KB - 1))
            nc.tensor.matmul(pvb[0:HD + 1, :],
                             lhsT=Vones[:, kb, 2 * g + 1, :],
                             rhs=pt[:, kb, 1, :],
                             start=(kb == 0), stop=(kb == KB - 1))

        def norm(g: int, qt: int, pva, pvb):
            qs = slice(qt * 512, (qt + 1) * 512)
            # stage PSUM->SBUF first so the PV banks free quickly
            stA = small.tile([HD + 1, 512], F32, tag="stA")
            nc.vector.tensor_copy(out=stA, in_=pva[0:HD + 1, :])
            stB = small.tile([HD + 1, 512], F32, tag="stB")
            nc.vector.tensor_copy(out=stB, in_=pvb[0:HD + 1, :])
            rs0 = small.tile([1, 2, 512], F32, tag="rs0")
            nc.vector.tensor_copy(out=rs0[0:1, 0, :], in_=stA[HD:HD + 1, :])
            nc.vector.tensor_copy(out=rs0[0:1, 1, :], in_=stB[HD:HD + 1, :])
            rsi = small.tile([1, 2, 512], F32, tag="rsi")
            nc.vector.reciprocal_approx_fast(out=rsi, in_=rs0)
            repA = small.tile([64, 512], F32, tag="repA")
            nc.gpsimd.partition_broadcast(out_ap=repA, in_ap=rsi[0:1, 0, :])
            repB = small.tile([64, 512], F32, tag="repB")
            nc.gpsimd.partition_broadcast(out_ap=repB, in_ap=rsi[0:1, 1, :])
            nc.vector.tensor_mul(out=aoT[0:64, g, qs], in0=stA[0:HD, :],
                                 in1=repA)
            nc.vector.tensor_mul(out=aoT[64:128, g, qs], in0=stB[0:HD, :],
                                 in1=repB)

        # ---------- filler queue (deadline-sorted) ----------
        # deadline = (window, slot) BEFORE which the item must be emitted.
        C_KQ, C_V, C_OP = 2.0, 1.2, 0.9
        items = []
        for qt in range(1, 4):
            items.append(((0, 4 * qt - 1), C_KQ,
                          lambda qt=qt: proj_kq("k", 0, qt)))
        for sb in range(SB):
            items.append(((0, sb), C_V, lambda sb=sb: proj_v(0, sb)))
        for qt in range(1, 4):
            items.append(((qt, 0), C_KQ, lambda qt=qt: proj_kq("q", 0, qt)))
        for g in range(1, 4):
            for qt in range(4):
                items.append(((4 * g, 0), C_KQ,
                              lambda g=g, qt=qt: proj_kq("k", g, qt)))
            items.append(((4 * g, 0), C_KQ,
                          lambda g=g: proj_kq("q", g, 0)))
            for qt in range(1, 4):
                items.append(((4 * g + qt, 0), C_KQ,
                              lambda g=g, qt=qt: proj_kq("q", g, qt)))
        for sb in range(SB):
            items.append(((8, sb), C_V, lambda sb=sb: proj_v(1, sb)))
        items.sort(key=lambda it: it[0])
        queue = list(items)

        def drain(w, j, opt_budget, spent):
            while queue:
                dl, cost, fn = queue[0]
                forced = dl <= (w, j)
                if not forced and spent[0] + cost > opt_budget:
                    break
                queue.pop(0)
                fn()
                spent[0] += cost

        # ---------- startup ----------
        proj_kq("k", 0, 0)
        proj_kq("q", 0, 0)

        # ---------- pipelined windows ----------
        # window w (= t): slots 0..15; slot j emits:
        #   fillers / outproj; PV(t-1, kb=j+8) for j<8 else PV(t, kb=j-8);
        #   scores+EXP for (t, kb=j)
        prev = None   # (g, qt, pt) of previous window
        pvcur = None  # (g, qt, pva, pvb) accumulation in flight
        # out-proj for qt becomes legal after norm(3, qt) at window 13+qt
        # slot 7; gate each item on (13+qt, 8).
        op_queue = [((13 + qt, 8), (qt, sb4, et))
                    for qt in range(3)
                    for et in range(2) for sb4 in range(4)]
        for w in range(16):
            g, qt = w // 4, w % 4
            pt = ptp.tile([128, KB, 2, 512], BF, tag="pt")
            fixed = 3.6 if prev is None else 10.6
            opt_budget = max(0.0, 16.6 - fixed)
            spent = [0.0]
            for j in range(16):
                if j < 8:
                    if prev is not None:
                        pv_slot(prev[0], j + 8, prev[2], pvcur[2], pvcur[3])
                        if j == 7:
                            norm(prev[0], prev[1], pvcur[2], pvcur[3])
                else:
                    if j == 8:
                        pva = psPV.tile([128, 512], F32, tag="pva")
                        pvb = psPV.tile([128, 512], F32, tag="pvb")
                        pvcur = (g, qt, pva, pvb)
                    pv_slot(g, j - 8, pt, pvcur[2], pvcur[3])
                slot_allow = opt_budget * (j + 1) / 16
                drain(w, j, slot_allow, spent)
                while (op_queue and op_queue[0][0] <= (w, j)
                       and spent[0] + C_OP <= slot_allow + 2.5):
                    _, o = op_queue.pop(0)
                    outproj_item(*o)
                    spent[0] += C_OP
                if j % 2 == 0:
                    scores_slot(g, qt, j, pt)
                    scores_slot(g, qt, j + 1, pt)
            prev = (g, qt, pt)

        # ---------- tail ----------
        for kb in range(8, 16):
            pv_slot(prev[0], kb, prev[2], pvcur[2], pvcur[3])
        norm(prev[0], prev[1], pvcur[2], pvcur[3])
        assert not queue
        for _, o in op_queue:
            outproj_item(*o)
        for et in range(2):
            for sb4 in range(4):
                outproj_item(3, sb4, et)

    nc.compile()
    return nc


def _prep_shards(x, Wq, bq, Wk, bk, Wv, bv, Wo):
    """Host-side shard prep. Returns per-core input maps (bf16)."""
    bf16 = ml_dtypes.bfloat16
    # host-side transpose into the device layout [128, EC, S]
    xs = [np.ascontiguousarray(
        x[b].T.reshape(EC, 128, S).transpose(1, 0, 2)).astype(bf16)
        for b in range(B)]
    halves = []
    for half in range(2):
        # Wq: scale folded in, columns duplicated per group
        wq_cols = (Wq[:, half * 256:(half + 1) * 256] * SCALE).reshape(E, HG, HD)
        wq_f = np.concatenate([wq_cols, wq_cols], axis=2).reshape(E, 512)
        bq_h = (bq[half * 256:(half + 1) * 256] * SCALE).reshape(HG, HD)
        qb_f = np.concatenate([bq_h, bq_h], axis=1).T.copy()  # [128, HG]

        wk_f = Wk[:, half * 512:(half + 1) * 512]
        kb_f = bk[half * 512:(half + 1) * 512].reshape(HG, 128).T.copy()

        wv_f = Wv[:, half * 512:(half + 1) * 512]
        vb_f = np.broadcast_to(
            bv[half * 512:(half + 1) * 512].reshape(1, 2, 256),
            (128, 2, 256)).copy()

        wo_f = Wo[half * 512:(half + 1) * 512, :]
        halves.append({
            "wq": np.ascontiguousarray(wq_f).astype(bf16),
            "wk": np.ascontiguousarray(wk_f).astype(bf16),
            "wv": np.ascontiguousarray(wv_f).astype(bf16),
            "wo": np.ascontiguousarray(wo_f).astype(bf16),
            "qb": qb_f.astype(np.float32),
            "kb": kb_f.astype(np.float32),
            "vb": vb_f.astype(np.float32),
        })
    in_maps = []
    for c in range(NCORES):
        m = {"x": xs[c // 2]}
        m.update(halves[c % 2])
        in_maps.append(m)
    return in_maps


def kernel(x, Wq, bq, Wk, bk, Wv, bv, Wo, bo):
    global LAST_RESULT
    x, Wq, bq, Wk, bk, Wv, bv, Wo, bo = [
        np.asarray(a, dtype=np.float32)
        for a in (x, Wq, bq, Wk, bk, Wv, bv, Wo, bo)]
    if "nc" not in _CACHE:
        _CACHE["nc"] = _build_program()
    nc = _CACHE["nc"]
    in_maps = _prep_shards(x, Wq, bq, Wk, bk, Wv, bv, Wo)
    res = run_bass_kernel_spmd(nc, in_maps, core_ids=list(range(NCORES)))
    LAST_RESULT = res
    out = np.empty((B, S, E), np.float32)
    for b in range(B):
        out[b] = res.results[2 * b]["out"] + res.results[2 * b + 1]["out"]
    out += bo.astype(np.float32)
    return out
